# revision 2
# baseline (speedup 1.0000x reference)
"""CACE GNN message-passing kernel for 8 trn2 NeuronCores.

Sharding: node-parallel. Edges are sorted by receiver and assigned to the core
owning the receiver node range (625 nodes/core). Each core:
  1. computes per-edge radial[8] / angular[20] / encoded[9] factors,
  2. scatter-adds rank-1 edge tensors into node buckets A[n,r,m,c] with ONE
     matmul per edge-chunk (lhsT = onehot(node)*radial [128e,112=(14n,8r)],
     rhs = ang x enc [128e,180=(20m,9c)]),
  3. evaluates the nu=2..4 symmetrization via a closed-form tensor-contraction
     plan (u=Sa, z=T:S, P=T:a, M=T:T, S2=S.S, ...) on DVE/GPSIMD/ACT.
No cross-core communication needed (pure node sharding).
"""
import math
import functools
import numpy as np

# ---------------- problem constants (hardcoded; must match reference) -------
N_NODES, N_EDGES = 5000, 50000
N_RBF, MAX_L = 8, 3
CUTOFF = 5.5
EPS = 1e-9
ZS = [1, 6, 7, 8]
N_CORES = 8
PER = N_NODES // N_CORES          # 625 nodes per core
NT = 14                           # nodes per chunk-tile
N_CH = 56                         # chunk-tiles per core (padded)
P = 128                           # edges per chunk (partitions)
NQ = NT * N_RBF                   # 112 = lhsT free
NM = 20                           # angular monomials
NC9 = 9                           # encoded channels
NF = 11                           # output features
SQ2C = math.sqrt(2.0 / CUTOFF)


def _lxlylz_list(max_l=3):
    lst = []
    for l in range(max_l + 1):
        for lx in range(l, -1, -1):
            for ly in range(l - lx, -1, -1):
                lst.append((lx, ly, l - lx - ly))
    return lst


LXLYLZ = _lxlylz_list()
IDX = {v: i for i, v in enumerate(LXLYLZ)}


def _e(i):
    v = [0, 0, 0]
    v[i] += 1
    return tuple(v)


def _vadd(*vs):
    o = [0, 0, 0]
    for v in vs:
        o = [o[k] + v[k] for k in range(3)]
    return tuple(o)


A_ = [IDX[_e(a)] for a in range(3)]
S_ = {(a, b): IDX[_vadd(_e(a), _e(b))] for a in range(3) for b in range(3)}
T_ = {(a, b, c): IDX[_vadd(_e(a), _e(b), _e(c))]
      for a in range(3) for b in range(3) for c in range(3)}
SYM2 = [(0, 0), (0, 1), (0, 2), (1, 1), (1, 2), (2, 2)]
W2 = {p: (1.0 if p[0] == p[1] else 2.0) for p in SYM2}
SYM3 = sorted({tuple(sorted(k)) for k in T_})


def _w3(t):
    cnt = {}
    for x in t:
        cnt[x] = cnt.get(x, 0) + 1
    r = math.factorial(3)
    for v in cnt.values():
        r //= math.factorial(v)
    return float(r)


# ---------------- symmetrization plan --------------------------------------
class _Plan:
    def __init__(self):
        self.ops = []
        self.nt = 0

    def t(self):
        self.nt += 1
        return ('t', self.nt - 1)

    def mul(self, a, b, eng='v'):
        d = self.t()
        self.ops.append((eng, 'mul', d, a, b))
        return d

    def add(self, a, b, eng='v'):
        d = self.t()
        self.ops.append((eng, 'add', d, a, b))
        return d

    def sq(self, a):
        d = self.t()
        self.ops.append(('s', 'sq', d, a))
        return d

    def wmul(self, a, w):
        d = self.t()
        self.ops.append(('s', 'wmul', d, a, float(w)))
        return d

    def dot(self, pairs, eng='v'):
        # sum w*Pa*Pb, grouping weights to minimize wmuls
        by_w = {}
        for (a, b, w) in pairs:
            by_w.setdefault(float(w), []).append((a, b))
        total = None
        for w, lst in sorted(by_w.items()):
            acc = None
            for (a, b) in lst:
                pr = self.mul(a, b, eng=eng)
                acc = pr if acc is None else self.add(acc, pr, eng=eng)
            if w != 1.0:
                acc = self.wmul(acc, w)
            total = acc if total is None else self.add(total, acc, eng=eng)
        return total


def build_plan():
    p = _Plan()
    A = lambda m: ('A', m)
    def SQ(m):
        return ('Q', m)

    # nu2: grouped-weight sums of squares
    def wsq_sum(items, eng='v'):
        by_w = {}
        for (m, w) in items:
            by_w.setdefault(float(w), []).append(m)
        total = None
        for w, ms in sorted(by_w.items()):
            acc = None
            for m in ms:
                acc = SQ(m) if acc is None else p.add(acc, SQ(m), eng=eng)
            if w != 1.0:
                acc = p.wmul(acc, w)
            total = acc if total is None else p.add(total, acc, eng=eng)
        return total

    nu2_1 = wsq_sum([(A_[a], 1.0) for a in range(3)])
    nu2_2 = wsq_sum([(S_[ab], W2[ab]) for ab in SYM2])
    nu2_3 = wsq_sum([(T_[t3], _w3(t3)) for t3 in SYM3])
    u = [p.dot([(A(S_[(a, b)]), A(A_[b]), 1.0) for b in range(3)]) for a in range(3)]
    z = [p.dot([(A(T_[tuple(sorted((a, b, c)))]), A(S_[(a, b)]), W2[(a, b)])
                for (a, b) in SYM2]) for c in range(3)]
    P2 = {bc: p.dot([(A(A_[a]), A(T_[tuple(sorted((a,) + bc))]), 1.0)
                     for a in range(3)]) for bc in SYM2}
    S2 = {ab: p.dot([(A(S_[(ab[0], k)]), A(S_[(k, ab[1])]), 1.0)
                     for k in range(3)], eng='g') for ab in SYM2}
    M = {cd: p.dot([(A(T_[tuple(sorted((a, b, cd[0])))]),
                     A(T_[tuple(sorted((a, b, cd[1])))]), W2[(a, b)])
                    for (a, b) in SYM2], eng='g') for cd in SYM2}
    trS3 = p.dot([(S2[ab], A(S_[ab]), W2[ab]) for ab in SYM2], eng='g')
    nu3_2 = p.dot([(M[cd], A(S_[cd]), W2[cd]) for cd in SYM2], eng='g')
    nu4_1 = p.dot([(u[a], u[a], 1.0) for a in range(3)])
    nu4_2 = p.dot([(u[a], z[a], 1.0) for a in range(3)])
    nu4_3 = p.dot([(P2[bc], P2[bc], W2[bc]) for bc in SYM2])
    nu4_5 = p.dot([(z[a], z[a], 1.0) for a in range(3)])
    feats = [nu2_1, nu2_2, nu2_3, trS3, nu3_2, nu4_1, nu4_2, nu4_3, nu4_2, nu4_5]
    for f, src in enumerate(feats):
        p.ops.append(('s', 'copy', ('F', f + 1), src))
    p.ops.append(('s', 'copy', ('F', 0), ('A', 0)))
    return p


def run_plan_numpy(plan, Ap):
    env = {('A', m): Ap[:, m] for m in range(20)}
    env.update({('Q', m): Ap[:, m] ** 2 for m in range(20)})
    F = np.zeros((Ap.shape[0], 11), Ap.dtype)
    for op in plan.ops:
        kind = op[1]
        dst = op[2]
        if kind == 'mul':
            v = env[op[3]] * env[op[4]]
        elif kind == 'add':
            v = env[op[3]] + env[op[4]]
        elif kind == 'sq':
            v = env[op[3]] ** 2
        elif kind == 'wmul':
            v = env[op[3]] * op[4]
        elif kind == 'copy':
            v = env[op[3]]
        if dst[0] == 'F':
            F[:, dst[1]] = v
        else:
            env[dst] = v
    return F


def _alloc_slots(plan):
    """linear-scan register allocation for scratch planes -> slot ids"""
    last_use = {}
    for i, op in enumerate(plan.ops):
        for x in op[3:]:
            if isinstance(x, tuple) and x[0] == 't':
                last_use[x] = i
    slot_of = {}
    free = []
    n_slots = 0
    for i, op in enumerate(plan.ops):
        dst = op[2]
        if dst[0] == 't':
            if free:
                slot_of[dst] = free.pop()
            else:
                slot_of[dst] = n_slots
                n_slots += 1
        for x in op[3:]:
            if isinstance(x, tuple) and x[0] == 't' and last_use.get(x) == i:
                free.append(slot_of[x])
    return slot_of, n_slots


# ---------------- device kernel build --------------------------------------
@functools.lru_cache(maxsize=2)
def _build_nc(debug=False):
    import concourse.bass as bass
    import concourse.bacc as bacc
    import concourse.mybir as mybir
    from concourse.tile import TileContext

    dt = mybir.dt.float32
    op_mult = mybir.AluOpType.mult
    op_add = mybir.AluOpType.add
    op_sub = mybir.AluOpType.subtract
    ACT = mybir.ActivationFunctionType

    nc = bacc.Bacc("TRN2", target_bir_lowering=False, debug=False,
                   num_devices=N_CORES)
    ed_d = nc.dram_tensor("ed", [P, N_CH * 12], dt, kind="ExternalInput")
    aux_d = nc.dram_tensor("aux", [P, N_CH + NQ + N_RBF], dt,
                           kind="ExternalInput")
    oh_d = nc.dram_tensor("oh8", [P, N_CH * NQ], mybir.dt.uint8,
                          kind="ExternalInput")
    out_d = nc.dram_tensor("out", [N_CH * NT, N_RBF * NF * NC9], dt,
                           kind="ExternalOutput")
    dbg = {}
    if debug:
        for nm, w in [("ang", N_CH * NM), ("radial", N_CH * N_RBF),
                      ("enc", N_CH * NC9), ("lhsT", N_CH * NQ),
                      ("A", N_CH * NM * NC9), ("ln", N_CH),
                      ("sinr", N_CH * N_RBF),
                      ("wfac", N_CH), ("fcv", N_CH)]:
            dbg[nm] = nc.dram_tensor("dbg_" + nm, [P, w], dt,
                                     kind="ExternalOutput")

    plan = build_plan()
    slot_of, n_slots = _alloc_slots(plan)

    with TileContext(nc) as tc:
        with (
            tc.tile_pool(name="io", bufs=1) as io,
            tc.tile_pool(name="apool", bufs=1) as apl,
            tc.tile_pool(name="psum", bufs=4, space="PSUM") as pp,
        ):
            ep_cm = tc.tile_pool(name="edge", bufs=1)
            ep = ep_cm.__enter__()
            ed = io.tile([P, N_CH * 12], dt)
            aux = io.tile([P, N_CH + NQ + N_RBF], dt)
            nc.sync.dma_start(out=ed[:, :], in_=ed_d[:, :])
            nc.sync.dma_start(out=aux[:, :], in_=aux_d[:, :])
            rloc = aux[:, 0:N_CH]
            cpat = aux[:, N_CH:N_CH + NQ]
            cn8 = aux[:, N_CH + NQ:N_CH + NQ + N_RBF]

            edv = ed[:, :].rearrange("p (ch t) -> p ch t", t=12)
            pos_s = edv[:, :, 0:3]
            pos_r = edv[:, :, 3:6]
            emb_s = edv[:, :, 6:9]
            emb_r = edv[:, :, 9:12]

            d = ep.tile([P, N_CH * 3], dt)
            dv = d[:, :].rearrange("p (ch t) -> p ch t", t=3)
            nc.vector.tensor_tensor(out=dv, in0=pos_r, in1=pos_s, op=op_sub)
            dsq = ep.tile([P, N_CH * 3], dt)
            dsqv = dsq[:, :].rearrange("p (ch t) -> p ch t", t=3)
            nc.vector.tensor_tensor(out=dsqv, in0=dv, in1=dv, op=op_mult)
            l2 = ep.tile([P, N_CH], dt)
            nc.vector.tensor_reduce(out=l2[:, :], in_=dsqv,
                                    axis=mybir.AxisListType.X, op=op_add)
            ln = ep.tile([P, N_CH], dt)
            nc.scalar.activation(out=ln[:, :], in_=l2[:, :], func=ACT.Sqrt)
            le = ep.tile([P, N_CH], dt)
            nc.vector.tensor_scalar_add(le[:, :], ln[:, :], EPS)
            rinv = ep.tile([P, N_CH], dt)
            nc.vector.reciprocal(out=rinv[:, :], in_=le[:, :])
            unit = ep.tile([P, N_CH * 3], dt)
            unitv = unit[:, :].rearrange("p (ch t) -> p ch t", t=3)
            nc.vector.tensor_tensor(
                out=unitv, in0=dv,
                in1=rinv[:, :].unsqueeze(2).to_broadcast([P, N_CH, 3]),
                op=op_mult)

            # angular monomials [p, ch, 20]
            ang = ep.tile([P, N_CH * NM], dt)
            av = ang[:, :].rearrange("p (ch m) -> p ch m", m=NM)
            nc.vector.memset(av[:, :, 0:1], 1.0)
            nc.vector.tensor_copy(av[:, :, 1:4], unitv)
            nc.vector.tensor_tensor(
                out=av[:, :, 4:7],
                in0=av[:, :, 1:2].to_broadcast([P, N_CH, 3]),
                in1=av[:, :, 1:4], op=op_mult)
            nc.vector.tensor_tensor(
                out=av[:, :, 7:9],
                in0=av[:, :, 2:3].to_broadcast([P, N_CH, 2]),
                in1=av[:, :, 2:4], op=op_mult)
            nc.vector.tensor_tensor(
                out=av[:, :, 9:10], in0=av[:, :, 3:4], in1=av[:, :, 3:4],
                op=op_mult)
            nc.vector.tensor_tensor(
                out=av[:, :, 10:16],
                in0=av[:, :, 1:2].to_broadcast([P, N_CH, 6]),
                in1=av[:, :, 4:10], op=op_mult)
            nc.vector.tensor_tensor(
                out=av[:, :, 16:19],
                in0=av[:, :, 2:3].to_broadcast([P, N_CH, 3]),
                in1=av[:, :, 7:10], op=op_mult)
            nc.vector.tensor_tensor(
                out=av[:, :, 19:20], in0=av[:, :, 3:4], in1=av[:, :, 9:10],
                op=op_mult)

            # encoded [p, ch, 3, 3] (c = s*3 + r)
            enc = ep.tile([P, N_CH * NC9], dt)
            ev = enc[:, :].rearrange("p (ch a b) -> p ch a b", a=3, b=3)
            nc.vector.tensor_tensor(
                out=ev,
                in0=emb_s.unsqueeze(3).to_broadcast([P, N_CH, 3, 3]),
                in1=emb_r.unsqueeze(2).to_broadcast([P, N_CH, 3, 3]),
                op=op_mult)

            # radial [p, ch, 8] via Chebyshev recurrence on clamped angle
            lc = ep.tile([P, N_CH], dt)
            nc.vector.tensor_scalar_min(lc[:, :], ln[:, :], CUTOFF)
            th = ep.tile([P, N_CH], dt)
            nc.vector.tensor_scalar_mul(th[:, :], lc[:, :], math.pi / CUTOFF)
            hh = ep.tile([P, N_CH], dt)
            nc.vector.tensor_scalar_mul(hh[:, :], lc[:, :],
                                        math.pi / (2.0 * CUTOFF))
            s2 = ep.tile([P, N_CH], dt)
            nc.scalar.activation(out=s2[:, :], in_=hh[:, :], func=ACT.Sin)
            s2q = ep.tile([P, N_CH], dt)
            nc.scalar.activation(out=s2q[:, :], in_=s2[:, :], func=ACT.Square)
            c2 = ep.tile([P, N_CH], dt)
            nc.vector.tensor_scalar(c2[:, :], s2q[:, :], -4.0, 2.0,
                                    op_mult, op_add)
            sinr = ep.tile([P, N_CH * N_RBF], dt)
            sv = sinr[:, :].rearrange("p (ch r) -> p ch r", r=N_RBF)
            nc.scalar.activation(out=sv[:, :, 0], in_=th[:, :], func=ACT.Sin)
            nc.vector.tensor_tensor(out=sv[:, :, 1], in0=c2[:, :],
                                    in1=sv[:, :, 0], op=op_mult)
            for n in range(2, N_RBF):
                tmp_n = ep.tile([P, N_CH], dt, tag=f"cheb{n % 2}")
                nc.vector.tensor_tensor(out=tmp_n[:, :], in0=c2[:, :],
                                        in1=sv[:, :, n - 1], op=op_mult)
                nc.vector.tensor_tensor(out=sv[:, :, n], in0=tmp_n[:, :],
                                        in1=sv[:, :, n - 2], op=op_sub)
            # fc polynomial
            uu = ep.tile([P, N_CH], dt)
            nc.vector.tensor_scalar_mul(uu[:, :], ln[:, :], 1.0 / CUTOFF)
            u2 = ep.tile([P, N_CH], dt)
            nc.vector.tensor_tensor(out=u2[:, :], in0=uu[:, :], in1=uu[:, :],
                                    op=op_mult)
            u3 = ep.tile([P, N_CH], dt)
            nc.vector.tensor_tensor(out=u3[:, :], in0=u2[:, :], in1=uu[:, :],
                                    op=op_mult)
            u6 = ep.tile([P, N_CH], dt)
            nc.vector.tensor_tensor(out=u6[:, :], in0=u3[:, :], in1=u3[:, :],
                                    op=op_mult)
            t1 = ep.tile([P, N_CH], dt)
            nc.vector.tensor_scalar(t1[:, :], uu[:, :], -21.0, 48.0,
                                    op_mult, op_add)
            t2 = ep.tile([P, N_CH], dt)
            nc.vector.tensor_tensor(out=t2[:, :], in0=t1[:, :], in1=uu[:, :],
                                    op=op_mult)
            nc.vector.tensor_scalar_add(t2[:, :], t2[:, :], -28.0)
            fcv = ep.tile([P, N_CH], dt)
            nc.vector.tensor_tensor(out=fcv[:, :], in0=u6[:, :], in1=t2[:, :],
                                    op=op_mult)
            nc.vector.tensor_scalar_add(fcv[:, :], fcv[:, :], 1.0)
            msk = ep.tile([P, N_CH], dt)
            nc.vector.tensor_scalar(msk[:, :], ln[:, :], CUTOFF, None,
                                    mybir.AluOpType.is_lt)
            nc.vector.tensor_tensor(out=fcv[:, :], in0=fcv[:, :], in1=msk[:, :],
                                    op=op_mult)
            wfac = ep.tile([P, N_CH], dt)
            nc.vector.tensor_tensor(out=wfac[:, :], in0=fcv[:, :],
                                    in1=rinv[:, :], op=op_mult)
            nc.vector.tensor_scalar_mul(wfac[:, :], wfac[:, :], SQ2C)
            radial = ep.tile([P, N_CH * N_RBF], dt)
            radv = radial[:, :].rearrange("p (ch r) -> p ch r", r=N_RBF)
            nc.vector.tensor_tensor(
                out=radv, in0=sinr[:, :].rearrange("p (ch r) -> p ch r", r=N_RBF),
                in1=wfac[:, :].unsqueeze(2).to_broadcast([P, N_CH, N_RBF]),
                op=op_mult)

            # rhs slab [p, ch, 20m, 9c] ; lhsT slab [p, ch, 14n, 8r]
            rhs = ep.tile([P, N_CH * NM * NC9], dt)
            rv = rhs[:, :].rearrange("p (ch m c) -> p ch m c", m=NM, c=NC9)
            nc.vector.tensor_tensor(
                out=rv,
                in0=av.unsqueeze(3).to_broadcast([P, N_CH, NM, NC9]),
                in1=ev.rearrange("p ch a b -> p ch (a b)").unsqueeze(2)
                    .to_broadcast([P, N_CH, NM, NC9]),
                op=op_mult)
            ohf = ep.tile([P, N_CH * NQ], dt)
            nc.gpsimd.dma_start(out=ohf[:, :], in_=oh_d[:, :])
            lhsT = ep.tile([P, N_CH * NQ], dt)
            lv = lhsT[:, :].rearrange("p (ch n r) -> p ch n r", n=NT, r=N_RBF)
            nc.vector.tensor_tensor(
                out=lv,
                in0=ohf[:, :].rearrange("p (ch n r) -> p ch n r", n=NT,
                                        r=N_RBF),
                in1=radv.unsqueeze(2).to_broadcast([P, N_CH, NT, N_RBF]),
                op=op_mult)

            # scatter matmuls -> A slab [112, ch*180]
            A = apl.tile([P, N_CH * NM * NC9], dt)
            lhv = lhsT[:, :].rearrange("p (ch q) -> p ch q", q=NQ)
            rhv = rhs[:, :].rearrange("p (ch f) -> p ch f", f=NM * NC9)
            Avw = A[:, :].rearrange("p (ch f) -> p ch f", f=NM * NC9)
            for ch2 in range(N_CH // 2):
                pt = pp.tile([NQ, 2 * NM * NC9], dt)
                for k in range(2):
                    ch = ch2 * 2 + k
                    nc.tensor.matmul(
                        out=pt[:, k * 180:(k + 1) * 180],
                        lhsT=lhv[:, ch, :], rhs=rhv[:, ch, :],
                        start=True, stop=True)
                nc.scalar.copy(
                    out=Avw[:NQ, ch2 * 2:ch2 * 2 + 2, :].rearrange(
                        "p ch f -> p (ch f)"),
                    in_=pt[:, :])

            if debug:
                nc.sync.dma_start(out=dbg["ang"][:, :], in_=ang[:, :])
                nc.sync.dma_start(out=dbg["radial"][:, :], in_=radial[:, :])
                nc.sync.dma_start(out=dbg["enc"][:, :], in_=enc[:, :])
                nc.sync.dma_start(out=dbg["lhsT"][:, :], in_=lhsT[:, :])
                nc.sync.dma_start(out=dbg["A"][:, :], in_=A[:, :])
                nc.sync.dma_start(out=dbg["ln"][:, :], in_=ln[:, :])
                nc.sync.dma_start(out=dbg["sinr"][:, :], in_=sinr[:, :])
                nc.sync.dma_start(out=dbg["wfac"][:, :], in_=wfac[:, :])
                nc.sync.dma_start(out=dbg["fcv"][:, :], in_=fcv[:, :])
            # ---- symmetrization ----
            ep_cm.__exit__(None, None, None)
            sy_cm = tc.tile_pool(name="sym", bufs=1)
            sy = sy_cm.__enter__()
            feats = sy.tile([P, N_CH * NF * NC9], dt)
            Qs = sy.tile([P, N_CH * NM * NC9], dt)
            nc.scalar.activation(out=Qs[:NQ, :], in_=A[:NQ, :],
                                 func=ACT.Square)
            slots = sy.tile([P, n_slots * N_CH * NC9], dt)
            slotv = slots[:, :].rearrange("p (s ch c) -> p s ch c", s=n_slots,
                                          c=NC9)

            def plane(pid):
                if pid[0] == 'A':
                    m = pid[1]
                    return A[:NQ, :].rearrange(
                        "p (ch m c) -> p ch m c", m=NM, c=NC9)[:, :, pid[1], :]
                if pid[0] == 'Q':
                    return Qs[:NQ, :].rearrange(
                        "p (ch m c) -> p ch m c", m=NM, c=NC9)[:, :, pid[1], :]
                if pid[0] == 'F':
                    return feats[:NQ, :].rearrange(
                        "p (ch f c) -> p ch f c", f=NF, c=NC9)[:, :, pid[1], :]
                return slotv[:NQ, slot_of[pid], :, :]

            eng_tt = {'v': nc.vector, 'g': nc.vector}
            for op in plan.ops:
                eng, kind, dst = op[0], op[1], op[2]
                do = plane(dst)
                if kind in ('mul', 'add'):
                    nc_e = eng_tt.get(eng, nc.vector)
                    nc_e.tensor_tensor(out=do, in0=plane(op[3]),
                                       in1=plane(op[4]),
                                       op=op_mult if kind == 'mul' else op_add)
                elif kind == 'sq':
                    nc.scalar.activation(out=do, in_=plane(op[3]),
                                         func=ACT.Square)
                elif kind == 'wmul':
                    nc.scalar.activation(out=do, in_=plane(op[3]),
                                         func=ACT.Copy, scale=float(op[4]))
                elif kind == 'copy':
                    nc.scalar.copy(out=do, in_=plane(op[3]))

            # output DMA: feats [112=(14n,8r), ch*(11f*9c)] -> [ch*14, 792]
            src = feats[:NQ, :].rearrange("p (ch x) -> p ch x", x=NF * NC9)
            dst = out_d[:, :].rearrange("(ch n) (r x) -> n r ch x",
                                        ch=N_CH, r=N_RBF)
            nc.sync.dma_start(out=dst, in_=src)
            sy_cm.__exit__(None, None, None)
    nc.compile()
    return nc, plan


# ---------------- host side -------------------------------------------------
def _host_prep(inputs):
    pos = np.ascontiguousarray(inputs['positions'], np.float32)
    W = np.asarray(inputs['W_embed'], np.float32)
    an = np.asarray(inputs['atomic_numbers'])
    ei = np.asarray(inputs['edge_index'])
    zs = np.asarray(ZS, an.dtype)
    onehot = (an[:, None] == zs[None, :]).astype(np.float32)
    emb = onehot @ W
    send, recv = ei[0], ei[1]
    order = np.argsort(recv, kind='stable')
    send, recv = send[order], recv[order]
    counts = np.bincount(recv, minlength=N_NODES)
    starts = np.concatenate([[0], np.cumsum(counts)])
    in_maps = []
    chunk_meta = []
    cpat = np.repeat(np.arange(NT, dtype=np.float32), N_RBF)[None, :].repeat(P, 0)
    cn8 = (np.arange(1, N_RBF + 1, dtype=np.float32) * np.pi / CUTOFF
           )[None, :].repeat(P, 0)
    for core in range(N_CORES):
        n0, n1 = core * PER, (core + 1) * PER
        chunks = []
        node = n0
        while node < n1:
            base = node
            e_lo = starts[node]
            while (node < n1 and node - base < NT
                   and starts[node + 1] - e_lo <= P):
                node += 1
            assert node > base, f"node {base} degree > {P}"
            chunks.append((int(e_lo), int(starts[node]), int(base)))
        assert len(chunks) <= N_CH, f"core {core}: {len(chunks)} chunks > {N_CH}"
        ed = np.zeros((P, N_CH, 12), np.float32)
        rloc = np.zeros((P, N_CH), np.float32)
        for ci, (lo, hi, base) in enumerate(chunks):
            k = hi - lo
            es, er = send[lo:hi], recv[lo:hi]
            ed[:k, ci, 0:3] = pos[es]
            ed[:k, ci, 3:6] = pos[er]
            ed[:k, ci, 6:9] = emb[es]
            ed[:k, ci, 9:12] = emb[er]
            rloc[:k, ci] = (er - base).astype(np.float32)
        aux = np.concatenate([rloc, cpat, cn8], axis=1)
        oh8 = (rloc[:, :, None] ==
               np.floor(np.arange(NQ, dtype=np.float32) / N_RBF)[None, None, :]
               ).astype(np.uint8)
        in_maps.append({
            "ed": np.ascontiguousarray(ed.reshape(P, N_CH * 12)),
            "aux": np.ascontiguousarray(aux),
            "oh8": np.ascontiguousarray(oh8.reshape(P, N_CH * NQ)),
        })
        chunk_meta.append(chunks)
    return in_maps, chunk_meta


LAST = {}


def kernel(**inputs):
    import os
    from concourse.bass_utils import run_bass_kernel_spmd
    nc, _plan = _build_nc()
    in_maps, chunk_meta = _host_prep(inputs)
    trace = bool(int(os.environ.get("KTRACE", "0")))
    res = run_bass_kernel_spmd(nc, in_maps, core_ids=list(range(N_CORES)),
                               trace=trace)
    LAST['res'] = res
    out = np.zeros((N_NODES, N_RBF, NF, NC9), np.float32)
    for core in range(N_CORES):
        slab = res.results[core]["out"].reshape(N_CH, NT, N_RBF, NF, NC9)
        n0, n1 = core * PER, (core + 1) * PER
        chunks = chunk_meta[core]
        for ci, (lo, hi, base) in enumerate(chunks):
            nxt = chunks[ci + 1][2] if ci + 1 < len(chunks) else n1
            out[base:nxt] = slab[ci, :nxt - base]
    return out

